# revision 29
# baseline (speedup 1.0000x reference)
"""MetaNet (2-layer GNN message passing) on 8 Trainium2 NeuronCores.

Sharding: edges sorted by destination node; nodes (and their incoming
edges) partitioned into 8 contiguous ranges, one per core. scatter_mean
is done transposed per node-tile: aggT[h, n] += me_k.T @ selT_k with the
mean's 1/cnt folded into the me evacuation and the post-aggregation bias
applied as a K=1 outer-product matmul. The edge->node MLP chain is
algebraically fused (e_w2 @ n?a_w1 folded) so no per-edge concat copies
are needed; x1 is AllGathered in 4 segments between the two layers.
"""

import sys

sys.path.insert(0, "/opt/trn_rl_repo")

import numpy as np

import concourse.bass as bass
import concourse.mybir as mybir
import concourse.tile as tile
from concourse.bass_utils import run_bass_kernel_spmd

F32 = mybir.dt.float32
BF16 = mybir.dt.bfloat16
I32 = mybir.dt.int32
P = 128
CW = 512   # edges per compute chunk (PSUM bank width)
SW = 1024  # edges per DMA superchunk
NCORES = 8
import os as _os0
GATHER_BATCH = int(_os0.environ.get("BASS_GATHER_BATCH", "1"))
ACT = mybir.ActivationFunctionType
AOP = mybir.AluOpType


def _r(ap):
    return ap.bitcast(mybir.dt.float32r)


def _split_multi_waits(nc):
    # This container's walrus build accepts only ONE sync-wait command per
    # instruction. Hoist extra waits onto same-engine NOPs placed directly
    # before the instruction (sequencers run in order, so semantics match).
    n = 0
    for bb in nc.main_func.blocks:
        new_insts = []
        for ins in bb.instructions:
            si = getattr(ins, "sync_info", None)
            if si is not None and si.on_wait and len(si.on_wait) > 1:
                waits = list(si.on_wait)
                for w in waits[:-1]:
                    nop = mybir.InstNoOp(
                        name=f"wsplit_{n}",
                        engine=ins.engine,
                        bass_nofuse=True,
                        sync_info=mybir.SyncInfo(on_wait=[w], on_update=[]),
                    )
                    n += 1
                    new_insts.append(nop)
                si.on_wait = [waits[-1]]
            new_insts.append(ins)
        bb.instructions[:] = new_insts
    return n


def _host_prep(x, edge_attr, edge_index):
    N = x.shape[0]
    npc = ((N + NCORES - 1) // NCORES + P - 1) // P * P  # nodes/core, mult of 128
    NT = npc // P
    npad = npc * NCORES

    row0 = edge_index[0].astype(np.int64)
    col0 = edge_index[1].astype(np.int64)

    # Remap nodes within each core so per-tile in-degree sums are flat
    # (LPT bin packing) — minimizes the uniform per-tile subchunk count
    # k128_u and with it the padded edge count E_pad.
    deg = np.bincount(row0, minlength=npad)
    nodemap = np.empty(npad, np.int64)
    Ec_all = [int(deg[c * npc:(c + 1) * npc].sum()) for c in range(NCORES)]
    kbase = max(1, max(Ec_all) // (NT * P))
    g_cap = kbase * P
    g_nover = min(NT, max(-(-(Ec - NT * g_cap) // P) for Ec in Ec_all))
    g_nover = max(g_nover, 0)
    for c in range(NCORES):
        dl = deg[c * npc:(c + 1) * npc]
        order_d = np.argsort(-dl, kind="stable")
        # serpentine deal of degree-sorted nodes across tiles
        tiles = [[] for _ in range(NT)]
        for j, i in enumerate(order_d):
            r, q = divmod(j, NT)
            t = q if (r % 2 == 0) else NT - 1 - q
            tiles[t].append(i)
        loads = np.array([int(dl[m].sum()) for m in [np.array(ti) for ti in tiles]])
        # concentrate the above-budget excess into a few overflow tiles so
        # the cross-core max of per-tile subchunk counts stays minimal
        cap = g_cap
        nover = g_nover
        hicap = cap + P
        over = list(range(NT - nover, NT)) if nover else []
        for _ in range(16 * NT):
            heavy = [t for t in range(NT - nover) if loads[t] > cap]
            if not heavy:
                break
            th = heavy[int(np.argmax(loads[heavy]))]
            cands = [t for t in over if loads[t] < hicap] or                     [t for t in range(NT - nover) if loads[t] < cap]
            if not cands:
                break
            tl = cands[int(np.argmin(loads[cands]))]
            tcap = hicap if tl in over else cap
            ai = int(np.argmax(dl[tiles[th]]))
            bi = int(np.argmin(dl[tiles[tl]]))
            a, b = tiles[th][ai], tiles[tl][bi]
            d = int(dl[a]) - int(dl[b])
            if d <= 0 or loads[tl] + d > tcap:
                # pick the largest move that still fits tl's cap
                room = tcap - loads[tl]
                degs_th = dl[tiles[th]]
                ok = np.nonzero(degs_th - int(dl[b]) <= room)[0]
                if len(ok) == 0:
                    break
                ai = int(ok[np.argmax(degs_th[ok])])
                a = tiles[th][ai]
                d = int(dl[a]) - int(dl[b])
                if d <= 0:
                    break
            tiles[th][ai], tiles[tl][bi] = b, a
            loads[th] -= d
            loads[tl] += d
        # relabel tiles by ascending load so per-index maxima align
        for newt, t in enumerate(np.argsort(loads, kind="stable")):
            for slot, i in enumerate(tiles[t]):
                nodemap[c * npc + i] = c * npc + newt * P + slot
    row = nodemap[row0]
    col = nodemap[col0]

    order = np.argsort(row, kind="stable")
    row_s, col_s = row[order], col[order]
    core_of = row_s // npc
    ltile = (row_s % npc) // P

    cnt_ct = np.zeros((NCORES, NT), np.int64)
    np.add.at(cnt_ct, (core_of, ltile), 1)
    k128_u = ((cnt_ct + P - 1) // P).max(axis=0)  # uniform subchunks per tile

    E_pad = int(k128_u.sum()) * P

    rowrel = np.full((NCORES, E_pad), -1, np.int32)
    rowglob = np.zeros((NCORES, E_pad), np.int32)
    colg = np.zeros((NCORES, E_pad), np.int32)
    ea_perm = np.zeros((NCORES, E_pad), np.int64)
    ea_valid = np.zeros((NCORES, E_pad), bool)
    tstart = np.concatenate([[0], np.cumsum(k128_u) * P])[:-1]

    for c in range(NCORES):
        idx_c = np.nonzero(core_of == c)[0]
        lt_c = ltile[idx_c]
        ord_lt = np.argsort(lt_c, kind="stable")
        idx_c = idx_c[ord_lt]
        lt_sorted = lt_c[ord_lt]
        starts = np.searchsorted(lt_sorted, np.arange(NT))
        ends = np.searchsorted(lt_sorted, np.arange(NT), side="right")
        for t in range(NT):
            m = idx_c[starts[t]:ends[t]]
            n = len(m)
            if n == 0:
                continue
            # sort this tile's edges by col for gather locality
            m = m[np.argsort(col_s[m], kind="stable")]
            o = int(tstart[t])
            rowrel[c, o:o + n] = (row_s[m] % P).astype(np.int32)
            rowglob[c, o:o + n] = row_s[m].astype(np.int32)
            colg[c, o:o + n] = col_s[m].astype(np.int32)
            ea_perm[c, o:o + n] = order[m]
            ea_valid[c, o:o + n] = True

    FE = edge_attr.shape[1]
    Fx = x.shape[1]
    x_full = np.zeros((npad, Fx), np.float32)
    x_full[nodemap[:N]] = x
    # layer-1 edge-input stream, rows ordered [x_col; x_row; edge_attr]
    ein1 = np.zeros((NCORES, 2 * Fx + FE, E_pad), np.float32)
    for c in range(NCORES):
        v = ea_valid[c]
        ein1[c][:Fx] = x_full[colg[c]].T
        ein1[c][Fx:2 * Fx, v] = x_full[rowglob[c][v]].T
        ein1[c][2 * Fx:][:, v] = edge_attr[ea_perm[c][v]].T

    # superchunk metadata: per tile, DMA groups of <=SW edges, each split
    # into <=CW compute pieces: (tile, sbase, sww, [(ebase, poff, w, isl, cid)])
    super_meta = []
    ci = 0
    for t in range(NT):
        width = int(k128_u[t]) * P
        sbase = int(tstart[t])
        off = 0
        while off < width:
            sww = min(SW, width - off)
            pieces = []
            poff = 0
            while poff < sww:
                w = min(CW, sww - poff)
                pieces.append((sbase + off + poff, poff, w,
                               (off + poff + w) == width, ci))
                ci += 1
                poff += w
            super_meta.append((t, sbase + off, sww, pieces))
            off += sww
    NCHUNK = ci

    # per-node counts -> per-edge inverse-count (mean folding) and node mask
    cnt = np.zeros(npad, np.int64)
    np.add.at(cnt, row, 1)
    inv = np.where(cnt > 0, 1.0 / np.maximum(cnt, 1), 0.0).astype(np.float32)
    msk = (cnt > 0).astype(np.float32)

    rowp4 = np.full((NCORES, max(NCHUNK, 1), P, 4), -1, np.float32)
    colp4 = np.zeros((NCORES, max(NCHUNK, 1), P, 4), np.int32)
    invp4 = np.zeros((NCORES, max(NCHUNK, 1), P, 4), np.float32)
    for (t, _sb, _sw, pieces) in super_meta:
      for (ebase, _po, w, _l, cid) in pieces:
        r = w // P
        for c in range(NCORES):
            rowp4[c, cid, :, :r] = rowrel[c, ebase:ebase + w].reshape(r, P).T
            colp4[c, cid, :, :r] = colg[c, ebase:ebase + w].reshape(r, P).T
            gl = rowglob[c, ebase:ebase + w].reshape(r, P).T
            vv = rowrel[c, ebase:ebase + w].reshape(r, P).T >= 0
            invp4[c, cid, :, :r] = np.where(vv, inv[gl], 0.0)

    rowp4b = rowp4.transpose(0, 2, 1, 3).reshape(NCORES, P, -1)
    colp4b = colp4.transpose(0, 2, 1, 3).reshape(NCORES, P, -1)
    invp4b = invp4.transpose(0, 2, 1, 3).reshape(NCORES, P, -1)

    return dict(N=N, npc=npc, NT=NT, npad=npad, NCHUNK=NCHUNK, E_pad=E_pad,
                super_meta=super_meta, rowrel=rowrel, nodemap=nodemap,
                rowp4b=rowp4b, colp4b=colp4b, invp4b=invp4b,
                msk=msk, ein1=ein1, x_full=x_full)


def kernel(x, edge_attr, edge_index, **wts):
    x = np.asarray(x, np.float32)
    edge_attr = np.asarray(edge_attr, np.float32)
    edge_index = np.asarray(edge_index)
    wts = {k: np.asarray(v, np.float32) for k, v in wts.items()}
    import os
    return _run(x, edge_attr, edge_index, wts,
                trace=os.environ.get("BASS_KERNEL_TRACE", "0") == "1")


def _bf(a):
    import ml_dtypes
    return np.asarray(a, dtype=ml_dtypes.bfloat16)


def _run(x, edge_attr, edge_index, wts, trace=False, build_only=False):
    pre = _host_prep(x, edge_attr, edge_index)
    F = x.shape[1]
    H = wts["e1_w2"].shape[1]
    FE = edge_attr.shape[1]
    npc, NT, NCHUNK, E_pad = pre["npc"], pre["NT"], pre["NCHUNK"], pre["E_pad"]

    # ---- algebraic folding (host) ----
    # layer1: hm = relu(xc@A[:F] + h1r@ (e1_w2@A[F:]) + (n1a_b1 + e1_b2@A[F:]))
    A1 = wts["n1a_w1"]
    Wc1 = wts["e1_w2"] @ A1[F:]
    b_hm1 = (wts["n1a_b1"] + wts["e1_b2"] @ A1[F:]).reshape(H, 1)
    # layer2 edge-mlp folds (ea2 never materialized)
    A2 = wts["n2a_w1"]
    Wc2 = wts["e2_w2"] @ A2[H:]
    b_hm2 = (wts["n2a_b1"] + wts["e2_b2"] @ A2[H:]).reshape(H, 1)
    b_h2 = (wts["e2_b1"] + wts["e1_b2"] @ wts["e2_w1"][2 * H:]).reshape(H, 1)
    FIN = 2 * F + FE
    # e1_w1 rows reordered to the [xc; xr; ea] stream layout; stacked with
    # the hm x-part (zero-padded below row F) as output columns H:2H
    e1w1 = np.concatenate([wts["e1_w1"][F:2 * F], wts["e1_w1"][:F],
                           wts["e1_w1"][2 * F:]])
    w1stack = np.concatenate(
        [e1w1, np.concatenate([A1[:F], np.zeros((FIN - F, H), np.float32)])],
        axis=1)
    w2stack = np.concatenate([wts["e2_w1"][H:2 * H], A2[:H]], axis=1)

    consts = dict(
        w1stack=w1stack, e1_b1=wts["e1_b1"].reshape(H, 1),
        e1_w2=_bf(wts["e1_w2"]),
        wc1=_bf(Wc1), b_hm1=b_hm1,
        mw2_1=_bf(wts["n1a_w2"]),
        n1b_wa=wts["n1b_w1"][F:].copy(), n1b_wx=wts["n1b_w1"][:F].copy(),
        n1b_b1=wts["n1b_b1"].reshape(H, 1),
        n1b_w2=wts["n1b_w2"], n1b_b2=wts["n1b_b2"].reshape(H, 1),
        w2r=wts["e2_w1"][:H].copy(), w2stack2=w2stack,
        w2e=_bf(wts["e1_w2"] @ wts["e2_w1"][2 * H:]), b_h2=b_h2,
        wc2=_bf(Wc2), b_hm2=b_hm2,
        mw2_2=_bf(wts["n2a_w2"]),
        n2b_wa=wts["n2b_w1"][H:].copy(), n2b_wx=wts["n2b_w1"][:H].copy(),
        n2b_b1=wts["n2b_b1"].reshape(H, 1),
        n2b_w2=wts["n2b_w2"],
        mb2_1=_bf(wts["n1a_b2"].reshape(1, H)),
        mb2_2=_bf(wts["n2a_b2"].reshape(1, H)),
        iotaF=_bf(np.tile(np.arange(P, dtype=np.float32)[None, :], (P, 1))),
        identb=_bf(np.eye(P, dtype=np.float32)),
        iotaP=np.arange(P, dtype=np.float32).reshape(P, 1),
        ident=np.eye(P, dtype=np.float32),
    )
    n2b_b2_val = float(wts["n2b_b2"].reshape(-1)[0])

    import concourse.bacc as bacc
    nc = bacc.Bacc(num_swdge_queues=int(_os0.environ.get("BASS_NSWQ", "4")))

    W_KEYS = {"w1stack", "n1b_wa", "n1b_wx", "n1b_w2",
              "w2r", "w2stack2", "n2b_wa", "n2b_wx", "n2b_w2"}
    BF_KEYS = {"mw2_1", "mw2_2", "w2e", "mb2_1", "mb2_2", "iotaF", "identb",
               "e1_w2", "wc1", "wc2"}
    F32R = mybir.dt.float32r
    dp = {}
    for k, v in consts.items():
        dt_ = F32R if k in W_KEYS else (BF16 if k in BF_KEYS else F32)
        dp[k] = nc.declare_dram_parameter(k, list(v.shape), dt_, isOutput=False)
    ein1_d = nc.declare_dram_parameter("ein1", [2 * F + FE, E_pad], F32R,
                                       isOutput=False)
    rowf_d = nc.declare_dram_parameter("rowflat", [E_pad], BF16, isOutput=False)
    NC4 = max(NCHUNK, 1) * 4
    rowp4_d = nc.declare_dram_parameter("rowp4b", [P, NC4], F32, isOutput=False)
    colp4_d = nc.declare_dram_parameter("colp4b", [P, NC4], I32, isOutput=False)
    invp4_d = nc.declare_dram_parameter("invp4b", [P, NC4], F32, isOutput=False)
    mskr_d = nc.declare_dram_parameter("mskrow", [1, npc], BF16, isOutput=False)
    xT_d = nc.declare_dram_parameter("xT_own", [F, npc], F32R, isOutput=False)
    x2_d = nc.declare_dram_parameter("x2", [npc, 4], F32, isOutput=True)
    assert NCHUNK * 4 * 14 <= 200 * 1024, "index tiles too big for SBUF"

    smeta = pre["super_meta"]
    import os as _os
    NSEG = int(_os.environ.get("BASS_NSEG", "8" if NT >= 8 else "2"))
    NSEG = max(1, min(NSEG, NT))
    seg_bounds = [NT * s // NSEG for s in range(NSEG + 1)]

    with tile.TileContext(nc) as tc:
        with (
            tc.tile_pool(name="cst", bufs=1) as cst,
            tc.tile_pool(name="sb", bufs=4) as sb,
            tc.tile_pool(name="ps", bufs=1, space="PSUM") as ps,
            tc.tile_pool(name="dram", bufs=1, space="DRAM") as dram,
        ):
            ct = {}
            for k, v in consts.items():
                dt_ = F32R if k in W_KEYS else (BF16 if k in BF_KEYS else F32)
                t_ = cst.tile(list(v.shape), dt_, name=f"c_{k}")
                nc.sync.dma_start(out=t_[:], in_=dp[k][:])
                ct[k] = t_
            r4all = cst.tile([P, NC4], F32, name="c_r4")
            nc.sync.dma_start(out=r4all[:], in_=rowp4_d[:])
            c4all = cst.tile([P, NC4], I32, name="c_c4")
            nc.sync.dma_start(out=c4all[:], in_=colp4_d[:])
            i4all = cst.tile([P, NC4], F32, name="c_i4")
            nc.sync.dma_start(out=i4all[:], in_=invp4_d[:])
            mskr = cst.tile([1, npc], BF16, name="c_mskr")
            nc.sync.dma_start(out=mskr[:], in_=mskr_d[:])
            x1T_all = cst.tile([H, npc], F32R, name="c_x1T")

            ea1T_d = dram.tile([H, E_pad], BF16, name="ea1T")
            x1own_d = dram.tile([npc, H], BF16, name="x1own")
            x1full_d = dram.tile([NCORES * npc, H], BF16, name="x1full")
            x1segs = []
            for s in range(NSEG):
                segw = (seg_bounds[s + 1] - seg_bounds[s]) * P
                x1segs.append(dram.tile([NCORES * segw, H], BF16,
                                        name=f"x1seg{s}", addr_space="Shared"))

            def scatter(me_sb, r4f, i4f, R, agg_ps, isl):
                # aggT[h, n] += me_k.T @ selT_k; selT carries 1/cnt (fused)
                for k in range(R):
                    selT = sb.tile([P, P], BF16, tag=f"selT{k}")
                    nc.vector.tensor_scalar(
                        out=selT[:], in0=ct["iotaF"][:],
                        scalar1=r4f[:, k:k + 1], scalar2=i4f[:, k:k + 1],
                        op0=AOP.is_equal, op1=AOP.mult)
                    nc.tensor.matmul(agg_ps[:], lhsT=me_sb[:, k * H:(k + 1) * H],
                                     rhs=selT[:], start=False,
                                     stop=(isl and k == R - 1),
                                     skip_group_check=True)

            def me_stage(mw2, hmr, me_ps, me_sb, R, on_act=False):
                # me (edge-major m values)
                for k in range(R):
                    nc.tensor.matmul(me_ps[:, k * H:(k + 1) * H],
                                     lhsT=hmr[:, k * P:(k + 1) * P],
                                     rhs=ct[mw2][:], start=True, stop=True)
                if on_act:
                    nc.scalar.activation(me_sb[:, :R * H], me_ps[:, :R * H],
                                         ACT.Copy)
                else:
                    nc.vector.tensor_copy(me_sb[:, :R * H], me_ps[:, :R * H])

            def node_mlp(t, aggT_ps, wa, wx, nb1, xT_rhs):
                aggT = sb.tile([H, P], F32R, tag="aggT")
                nc.vector.tensor_copy(aggT[:], aggT_ps[:])
                hn_ps = ps.tile([H, P], F32, tag="sm", bufs=2)
                nc.tensor.matmul(hn_ps[:], lhsT=_r(ct[wa][:]), rhs=_r(aggT[:]),
                                 start=True, stop=False)
                nc.tensor.matmul(hn_ps[:], lhsT=_r(ct[wx][:]), rhs=xT_rhs,
                                 start=False, stop=True)
                hn = sb.tile([H, P], F32, tag="hn_sb")
                nc.vector.tensor_scalar(
                    out=_r(hn[:]), in0=hn_ps[:], scalar1=ct[nb1][:, :1],
                    scalar2=0.0, op0=AOP.add, op1=AOP.max)
                return hn

            # ================= layer 1 =================
            def finish_tile1(t, agg_ps):
                # node mlp + x1 output for tile t (emitted one tile late so
                # the in-order engine queues never head-block on the wait
                # for the tile's scatter chain)
                xTt = sb.tile([F, P], F32R, tag="xTt")
                nc.sync.dma_start(out=xTt[:], in_=xT_d[:, t * P:(t + 1) * P])
                hn = node_mlp(t, agg_ps, "n1b_wa", "n1b_wx", "n1b_b1",
                              _r(xTt[:]))
                x1T_ps = ps.tile([H, P], F32, tag="sm", bufs=2)
                nc.tensor.matmul(x1T_ps[:], lhsT=_r(ct["n1b_w2"][:]),
                                 rhs=_r(hn[:]), start=True, stop=True)
                nc.vector.tensor_scalar(
                    out=x1T_all[:, t * P:(t + 1) * P], in0=x1T_ps[:],
                    scalar1=ct["n1b_b2"][:, :1], scalar2=0.0,
                    op0=AOP.add, op1=AOP.max)
                x1_ps = ps.tile([P, H], F32, tag="sm", bufs=2)
                nc.tensor.transpose(out=x1_ps[:],
                                    in_=x1T_all[:, t * P:(t + 1) * P].bitcast(F32),
                                    identity=ct["ident"][:H, :H])
                x1sb = sb.tile([P, H], BF16, tag="x1sb")
                nc.vector.tensor_copy(x1sb[:], x1_ps[:])
                nc.scalar.dma_start(out=x1own_d[t * P:(t + 1) * P, :],
                                    in_=x1sb[:])
                for s in range(NSEG):
                    if t == seg_bounds[s + 1] - 1:
                        lo, hi = seg_bounds[s] * P, seg_bounds[s + 1] * P
                        nc.gpsimd.collective_compute(
                            "AllGather", AOP.bypass,
                            replica_groups=[list(range(NCORES))],
                            ins=[x1own_d[lo:hi].opt()],
                            outs=[x1segs[s].rearrange(
                                "(c n) h -> c n h", c=NCORES).opt()])

            def copy_segments():
                # chained after the last L1 store so the scheduler cannot
                # hoist these (collective-waiting) copies into the middle of
                # the layer-1 Pool stream, which would head-block it
                if last_l1_store[0] is not None:
                    tc.chain_iter_dep("x1copy", last_l1_store[0])
                for s in range(NSEG):
                    lo, hi = seg_bounds[s] * P, seg_bounds[s + 1] * P
                    cp = nc.gpsimd.dma_start(
                        out=x1full_d.rearrange("(c n) h -> c n h",
                                               c=NCORES)[:, lo:hi],
                        in_=x1segs[s].rearrange("(c n) h -> c n h",
                                                c=NCORES))
                    tc.chain_iter_dep("x1copy", cp.ins)

            last_l1_store = [None]
            pending1 = None
            for t in range(NT):
                supers = [s for s in smeta if s[0] == t]
                nch = sum(len(s[3]) for s in supers)
                agg_ps = ps.tile([H, P], F32, tag="agg", bufs=2)
                # bias-mask outer product opens the accumulation group
                nc.tensor.matmul(agg_ps[:], lhsT=ct["mb2_1"][:],
                                 rhs=mskr[:, t * P:(t + 1) * P],
                                 start=True, stop=(nch == 0),
                                 skip_group_check=True)
                for si, (tt, sbase, sww, pieces) in enumerate(supers):
                    rhsF = sb.tile([2 * F + FE, SW], F32R, tag="rhsF")
                    nc.sync.dma_start(out=rhsF[:, :sww],
                                      in_=ein1_d[:, sbase:sbase + sww])
                    h1r = sb.tile([H, SW], BF16, tag="h1r")
                    for (ebase, poff, W, isl, cid) in pieces:
                        R = W // P
                        r4f = r4all[:, cid * 4:cid * 4 + 4]
                        i4f = i4all[:, cid * 4:cid * 4 + 4]
                        pf = slice(poff, poff + W)
                        # stacked: rows 0:H = h1 pre-act, H:2H = wxc1.T@xc
                        hh_ps = ps.tile([P, CW], F32, tag="h1hm", bufs=3)
                        nc.tensor.matmul(hh_ps[:, :W], lhsT=_r(ct["w1stack"][:]),
                                         rhs=rhsF[:, pf], start=True, stop=False)
                        nc.scalar.activation(h1r[:, pf], hh_ps[:H, :W],
                                             ACT.Relu, bias=ct["e1_b1"][:, :1])
                        # hm = relu(Wc1.T @ h1r + [stacked xc part] + b_hm1)
                        nc.tensor.matmul(hh_ps[H:2 * H, :W], lhsT=ct["wc1"][:],
                                         rhs=h1r[:, pf], start=False, stop=True)
                        hmr = sb.tile([H, CW], BF16, tag="hmr")
                        if cid % 3 == 2:  # keep ACT/DVE evac load balanced
                            nc.vector.tensor_scalar(
                                out=hmr[:, :W], in0=hh_ps[H:2 * H, :W],
                                scalar1=ct["b_hm1"][:, :1], scalar2=0.0,
                                op0=AOP.add, op1=AOP.max)
                        else:
                            nc.scalar.activation(hmr[:, :W], hh_ps[H:2 * H, :W],
                                                 ACT.Relu,
                                                 bias=ct["b_hm1"][:, :1])
                        me_ps = ps.tile([P, 4 * H], F32, tag="sm", bufs=2)
                        me_sb = sb.tile([P, 4 * H], BF16, tag="me_sb")
                        me_stage("mw2_1", hmr, me_ps, me_sb, R,
                                 on_act=(cid % 3 == 2))
                        scatter(me_sb, r4f, i4f, R, agg_ps, isl)
                    # store relu'd h1 (layer 2 rebuilds its ea term from it);
                    # issued on the (idle-in-L1) gpsimd queue so the waiting
                    # store never head-blocks SP's rhsF prefetch stream
                    st = nc.scalar.dma_start(out=ea1T_d[:, sbase:sbase + sww],
                                             in_=h1r[:, :sww])
                    last_l1_store[0] = st.ins
                    if si == 0 and pending1 is not None:
                        finish_tile1(*pending1)
                        pending1 = None
                if pending1 is not None:
                    finish_tile1(*pending1)
                pending1 = (t, agg_ps)
            if pending1 is not None:
                finish_tile1(*pending1)
            copy_segments()

            # ================= layer 2 =================
            def finish_tile2(t, agg_ps):
                hn = node_mlp(t, agg_ps, "n2b_wa", "n2b_wx", "n2b_b1",
                              _r(x1T_all[:, t * P:(t + 1) * P]))
                x2_ps = ps.tile([P, 4], F32, tag="sm", bufs=2)
                nc.tensor.matmul(x2_ps[:, :1], lhsT=hn[:],
                                 rhs=ct["n2b_w2"][:].bitcast(F32),
                                 start=True, stop=True)
                x2sb = sb.tile([P, 4], F32, tag="x2sb")
                nc.scalar.activation(x2sb[:, :1], x2_ps[:, :1], ACT.Copy,
                                     bias=n2b_b2_val)
                nc.sync.dma_start(out=x2_d[t * P:(t + 1) * P, :1],
                                  in_=x2sb[:, :1])

            pending2 = None
            for t in range(NT):
                supers = [s for s in smeta if s[0] == t]
                nch = sum(len(s[3]) for s in supers)
                # per-tile row-table: HrowT[n, h] = x1[n] @ w2r
                hrow_ps = ps.tile([P, H], F32, tag="sm", bufs=2)
                nc.tensor.matmul(hrow_ps[:], lhsT=x1T_all[:, t * P:(t + 1) * P],
                                 rhs=_r(ct["w2r"][:]), start=True, stop=True)
                hrowT = sb.tile([P, H], BF16, tag="hrowT")
                nc.vector.tensor_copy(hrowT[:], hrow_ps[:])
                agg_ps = ps.tile([H, P], F32, tag="agg", bufs=2)
                nc.tensor.matmul(agg_ps[:], lhsT=ct["mb2_2"][:],
                                 rhs=mskr[:, t * P:(t + 1) * P],
                                 start=True, stop=(nch == 0),
                                 skip_group_check=True)
                for si, (tt, sbase, sww, pieces) in enumerate(supers):
                    # row-index broadcast + ea1 stream for the superchunk
                    rb = sb.tile([P, SW], BF16, tag="rb")
                    nc.scalar.dma_start(
                        out=rb[:, :sww],
                        in_=rowf_d[None, sbase:sbase + sww].to_broadcast(
                            [P, sww]))
                    rhsB = sb.tile([H, SW], BF16, tag="rhsB")
                    nc.sync.dma_start(out=rhsB[:, :sww],
                                      in_=ea1T_d[:, sbase:sbase + sww])
                    for (ebase, poff, W, isl, cid) in pieces:
                        R = W // P
                        r4f = r4all[:, cid * 4:cid * 4 + 4]
                        i4f = i4all[:, cid * 4:cid * 4 + 4]
                        c4 = c4all[:, cid * 4:cid * 4 + 4]
                        pf = slice(poff, poff + W)
                        selN = sb.tile([P, CW], BF16, tag="selN")
                        nc.vector.tensor_scalar(
                            out=selN[:, :W], in0=rb[:, pf],
                            scalar1=ct["iotaP"][:, :1], scalar2=None,
                            op0=AOP.is_equal)
                        g = sb.tile([P, 4 * H], BF16, tag="g")
                        k = 0
                        while k < R:
                            gb = min(GATHER_BATCH, R - k)
                            nc.gpsimd.indirect_dma_start(
                                out=g[:, k * H:(k + gb) * H], out_offset=None,
                                in_=x1full_d[:],
                                in_offset=bass.IndirectOffsetOnAxis(
                                    ap=c4[:, k:k + gb], axis=0))
                            k += gb
                        tc_ps = ps.tile([H, CW], BF16, tag="eatc", bufs=1)
                        for k in range(R):
                            nc.tensor.transpose(
                                out=tc_ps[:, k * P:(k + 1) * P],
                                in_=g[:, k * H:(k + 1) * H],
                                identity=ct["identb"][:])
                        xcT = sb.tile([H, CW], F32R, tag="xcT")
                        nc.vector.tensor_copy(xcT[:, :W], tc_ps[:, :W])
                        # stacked: rows 0:H = w2c.T@xc (h1), H:2H = wxc2.T@xc
                        hh_ps = ps.tile([P, CW], F32, tag="h1hm", bufs=3)
                        nc.tensor.matmul(hh_ps[:, :W], lhsT=_r(ct["w2stack2"][:]),
                                         rhs=_r(xcT[:, :W]), start=True,
                                         stop=False)
                        nc.tensor.matmul(hh_ps[:H, :W], lhsT=hrowT[:],
                                         rhs=selN[:, :W], start=False, stop=False)
                        nc.tensor.matmul(hh_ps[:H, :W], lhsT=ct["w2e"][:],
                                         rhs=rhsB[:, pf], start=False, stop=False)
                        h1r = sb.tile([H, CW], BF16, tag="h1r")
                        nc.scalar.activation(h1r[:, :W], hh_ps[:H, :W],
                                             ACT.Relu, bias=ct["b_h2"][:, :1])
                        nc.tensor.matmul(hh_ps[H:2 * H, :W], lhsT=ct["wc2"][:],
                                         rhs=h1r[:, :W], start=False,
                                         stop=True)
                        hmr = sb.tile([H, CW], BF16, tag="hmr")
                        nc.scalar.activation(hmr[:, :W], hh_ps[H:2 * H, :W],
                                             ACT.Relu, bias=ct["b_hm2"][:, :1])
                        me_ps = ps.tile([P, 4 * H], F32, tag="sm", bufs=2)
                        me_sb = sb.tile([P, 4 * H], BF16, tag="me_sb")
                        me_stage("mw2_2", hmr, me_ps, me_sb, R)
                        scatter(me_sb, r4f, i4f, R, agg_ps, isl)
                    if si == 0 and pending2 is not None:
                        finish_tile2(*pending2)
                        pending2 = None
                if pending2 is not None:
                    finish_tile2(*pending2)
                pending2 = (t, agg_ps)
            if pending2 is not None:
                finish_tile2(*pending2)

    nc.compile()

    import ml_dtypes
    in_maps = []
    for c in range(NCORES):
        m = {k: v for k, v in consts.items()}
        m["ein1"] = pre["ein1"][c]
        m["rowflat"] = _bf(pre["rowrel"][c])
        m["rowp4b"] = pre["rowp4b"][c].astype(np.float32)
        m["colp4b"] = pre["colp4b"][c]
        m["invp4b"] = pre["invp4b"][c]
        m["mskrow"] = _bf(pre["msk"][c * npc:(c + 1) * npc].reshape(1, npc))
        m["xT_own"] = pre["x_full"][c * npc:(c + 1) * npc].T.copy()
        in_maps.append(m)

    kernel.last_nc = nc
    kernel.last_in_maps = in_maps
    if build_only:
        return pre
    r = run_bass_kernel_spmd(nc, in_maps, list(range(NCORES)), trace=trace)
    kernel.last_results = r
    out = np.concatenate([r.results[c]["x2"][:, :1] for c in range(NCORES)], axis=0)
    return out[pre["nodemap"][:pre["N"]]].astype(np.float32)

